# revision 6
# baseline (speedup 1.0000x reference)
"""Causal multi-head attention (B=4, S=2048, D=1024, H=16) on 8 TRN2 cores.

Sharding: core c -> (batch b = c//2, head-group g = c%2, 8 heads each).
Host pre-transposes/splits inputs; device returns per-core partial outputs
y_c = attn_heads(g) @ wo[g-rows]; host sums the two partials per batch.

v2 design (vs v1 baseline at 637us):
  - Q/K projections 2-pass (xhi@whi + xhi@wlo); x lo parts never loaded.
  - 1/sqrt(dk) folded into wq on host (kills the per-chunk scalar.mul).
  - QK^T single packed pass: [qhi;qlo] . [khi;khi] = (qhi+qlo)*khi.
  - Causal mask + row-max fused in one DVE tensor_tensor_reduce pass.
  - All phases interleaved (proj slab m+1 and O-proj overlap attention) with
    one long-lived pool scope; PSUM: sc 2x2 banks + proj/Oproj 2 + pv 2.
  - 2-chunk rows (klen>1024): chunk1 exp'd with its own max early (frees
    PSUM), then rescaled by w1=exp(m1-M) on GpSimd after transpose.
  Expected rel err ~1.26e-2 (host-verified on true inputs), gate 2e-2.
"""

import numpy as np

import concourse.bacc as bacc
import concourse.tile as tile
from concourse import mybir
from concourse.bass_utils import run_bass_kernel_spmd

B, S, D = 4, 2048, 1024
H, DK = 16, 64
HL = 8            # heads per core
DL = HL * DK      # 512 local channels
N_CORES = 8
P = 128           # partitions
MT = 4            # m-slabs of 512 seq positions
NT = DL // P      # 4 channel slabs of 128
KT = D // P       # 8 contraction tiles
QT = S // P       # 16 q tiles of 128
CHUNK = 1024      # score chunk (2 PSUM banks)
LAG = 4           # alpha -> beta pipeline distance

f32 = mybir.dt.float32
f16 = mybir.dt.float16
AX = mybir.AxisListType.X
ALU = mybir.AluOpType
AF = mybir.ActivationFunctionType

_cache = {}


def _build():
    nc = bacc.Bacc("TRN2", target_bir_lowering=False)

    def din(name, shape, dt=f16):
        return nc.dram_tensor(name, shape, dt, kind="ExternalInput").ap()

    xq16 = din("xq16", [D, S])
    xk16 = din("xk16", [D, S])
    xv16 = din("xv16", [D, S])
    wq_hi = din("wq_hi", [D, DL]); wq_lo = din("wq_lo", [D, DL])
    wk_hi = din("wk_hi", [D, DL]); wk_lo = din("wk_lo", [D, DL])
    wv16 = din("wv16", [D, DL])
    wo16 = din("wo16", [DL, D])
    maskp = din("maskp", [P, CHUNK], f32)   # zeros + upper-tri -1e30 in last 128
    y = nc.dram_tensor("y", [S, D], f32, kind="ExternalOutput").ap()

    with tile.TileContext(nc) as tc:
        _body(nc, tc, xq16, xk16, xv16,
              wq_hi, wq_lo, wk_hi, wk_lo, wv16, wo16, maskp, y)
    nc.compile()
    return nc


def _body(nc, tc, xq16, xk16, xv16,
          wq_hi, wq_lo, wk_hi, wk_lo, wv16, wo16, maskp, y):
    from contextlib import ExitStack
    ctx = ExitStack()
    with ctx:
        persist = ctx.enter_context(tc.tile_pool(name="persist", bufs=1))
        qpool = ctx.enter_context(tc.tile_pool(name="qpool", bufs=16))
        xpool = ctx.enter_context(tc.tile_pool(name="xpool", bufs=3))
        stage = ctx.enter_context(tc.tile_pool(name="stage", bufs=2))
        pcpool = ctx.enter_context(tc.tile_pool(name="pcpool", bufs=2))
        ptpool = ctx.enter_context(tc.tile_pool(name="ptpool", bufs=6))
        statp = ctx.enter_context(tc.tile_pool(name="statp", bufs=LAG + 6))
        ostgp = ctx.enter_context(tc.tile_pool(name="ostgp", bufs=2))
        outtp = ctx.enter_context(tc.tile_pool(name="outtp", bufs=3))
        ypool = ctx.enter_context(tc.tile_pool(name="ypool", bufs=2))
        scp = ctx.enter_context(tc.tile_pool(name="scp", bufs=2, space="PSUM"))
        mmp = ctx.enter_context(tc.tile_pool(name="mmp", bufs=2, space="PSUM"))
        pvp = ctx.enter_context(tc.tile_pool(name="pvp", bufs=2, space="PSUM"))

        # ---------- persistent tiles ----------
        ktx = [[persist.tile([P, 512], f16, tag=f"ktx_{h}_{m}", name=f"ktx_{h}_{m}")
                for m in range(MT)] for h in range(HL)]
        vsb = [persist.tile([P, DL], f16, tag=f"v_{mt}", name=f"v_{mt}") for mt in range(QT)]
        mask_sb = persist.tile([P, CHUNK], f32, tag="mask", name="mask_sb")
        nc.sync.dma_start(out=mask_sb, in_=maskp)

        def load_w(nm, dr):
            t = persist.tile([P, KT, DL], f16, tag=f"w_{nm}", name=f"w_{nm}")
            nc.sync.dma_start(out=t, in_=dr.rearrange("(k p) n -> p k n", p=P))
            return t

        wq_h = load_w("qh", wq_hi); wq_l = load_w("ql", wq_lo)
        wk_h = load_w("kh", wk_hi); wk_l = load_w("kl", wk_lo)
        wv_sb = load_w("v", wv16)
        wo_sb = persist.tile([P, NT, D], f16, tag="wo", name="wo_sb")
        nc.sync.dma_start(out=wo_sb, in_=wo16.rearrange("(j p) n -> p j n", p=P))

        qpk = {}          # (h, m) -> [P, 512] tile: rows 0:64 qhi, 64:128 qlo
        stats = {}        # (qt, h) -> stat tile [P, 8]
        ptt = {}          # (qt, h) -> pt tile
        ostg = {}         # qt -> [P, DL] f16

        # ---------- projection slab m (512 seq positions) ----------
        def proj_group(m):
            msl = slice(m * 512, (m + 1) * 512)
            xq = xpool.tile([P, KT, 512], f16, tag="xh", name=f"xq{m}")
            xk = xpool.tile([P, KT, 512], f16, tag="xh", name=f"xk{m}")
            xv = xpool.tile([P, KT, 512], f16, tag="xh", name=f"xv{m}")
            nc.gpsimd.dma_start(out=xq, in_=xq16.rearrange("(k p) s -> p k s", p=P)[:, :, msl])
            nc.gpsimd.dma_start(out=xk, in_=xk16.rearrange("(k p) s -> p k s", p=P)[:, :, msl])
            nc.gpsimd.dma_start(out=xv, in_=xv16.rearrange("(k p) s -> p k s", p=P)[:, :, msl])

            # Q: 2-pass -> split hi/lo -> qpk[h][m] = [qhi;qlo]
            for n in range(NT):
                ps = mmp.tile([P, 512], f32, tag="mm512", name=f"psq{m}{n}")
                i = 0
                for whl in (wq_h, wq_l):
                    for k in range(KT):
                        nc.tensor.matmul(ps[:], whl[:, k, n * P:(n + 1) * P], xq[:, k],
                                         start=(i == 0), stop=(i == 2 * KT - 1))
                        i += 1
                st_hi = stage.tile([P, 512], f16, tag="st_hi", name="st_hi")
                st_lo = stage.tile([P, 512], f16, tag="st_lo", name="st_lo")
                nc.scalar.copy(st_hi[:], ps[:])
                nc.vector.tensor_tensor(out=st_lo, in0=ps[:], in1=st_hi, op=ALU.subtract)
                for hh in range(2):
                    h = 2 * n + hh
                    t = qpool.tile([P, 512], f16, tag="qpk", name=f"qpk{h}_{m}")
                    qpk[(h, m)] = t
                    rsl = slice(hh * DK, hh * DK + DK)
                    nc.gpsimd.dma_start(out=t[0:DK, :], in_=st_hi[rsl, :])
                    nc.gpsimd.dma_start(out=t[DK:P, :], in_=st_lo[rsl, :])

            # K: 2-pass -> khi only, duplicated into both halves of ktx
            for n in range(NT):
                ps = mmp.tile([P, 512], f32, tag="mm512", name=f"psk{m}{n}")
                i = 0
                for whl in (wk_h, wk_l):
                    for k in range(KT):
                        nc.tensor.matmul(ps[:], whl[:, k, n * P:(n + 1) * P], xk[:, k],
                                         start=(i == 0), stop=(i == 2 * KT - 1))
                        i += 1
                st_hi = stage.tile([P, 512], f16, tag="st_hi", name="st_hi")
                nc.scalar.copy(st_hi[:], ps[:])
                for hh in range(2):
                    h = 2 * n + hh
                    rsl = slice(hh * DK, hh * DK + DK)
                    nc.gpsimd.dma_start(out=ktx[h][m][0:DK, :], in_=st_hi[rsl, :])
                    nc.gpsimd.dma_start(out=ktx[h][m][DK:P, :], in_=st_hi[rsl, :])

            # V: seq-major per 128-tile
            for mt in range(4):
                ps = mmp.tile([P, 512], f32, tag="mm512", name=f"psv{m}{mt}")
                for k in range(KT):
                    nc.tensor.matmul(ps[:], xv[:, k, mt * P:(mt + 1) * P], wv_sb[:, k],
                                     start=(k == 0), stop=(k == KT - 1))
                nc.scalar.copy(vsb[4 * m + mt][:], ps[:])

        # ---------- attention ----------
        def chunks_of(qt):
            klen = (qt + 1) * P
            if klen <= CHUNK:
                return [(0, klen)]
            return [(0, CHUNK), (CHUNK, klen)]

        def alpha(qt, h):
            klen = (qt + 1) * P
            chs = chunks_of(qt)
            st = statp.tile([P, 10], f32, tag="stat", name=f"st{qt}_{h}")
            stats[(qt, h)] = st
            # stat cols: 0=m1n 1=m2n 2=Mn 3=w1 4=z1 5=z2 6=rh 7=s1 8=z1w 9=z
            pc = pcpool.tile([P, klen], f16, tag="pc", padded_shape=[P, S], name="pc")
            lq = qpk[(h, qt // 4)][:, (qt % 4) * P:(qt % 4 + 1) * P]
            for ci, (c0, c1) in enumerate(chs):
                cl = c1 - c0
                sc = scp.tile([P, CHUNK], f32, tag="sc", name=f"sc{ci}")
                for n0 in range(0, cl, 512):
                    nn = min(512, cl - n0)
                    g = c0 + n0
                    rk = ktx[h][g // 512][:, g % 512:g % 512 + nn]
                    nc.tensor.matmul(sc[:, n0:n0 + nn], lq, rk, start=True, stop=True)
                if c1 == klen:  # diagonal block: causal mask (in-place add)
                    nc.vector.tensor_tensor(
                        out=sc[:, cl - P:cl], in0=sc[:, cl - P:cl],
                        in1=mask_sb[:, CHUNK - P:CHUNK], op=ALU.add)
                nc.vector.reduce_max(st[:, ci:ci + 1], sc[:, 0:cl],
                                     axis=AX, negate=True)
                if ci == 0:
                    # chunk1 exp with its own (negated) max as bias
                    nc.scalar.activation(pc[:, c0:c1], sc[:, 0:cl], AF.Exp,
                                         bias=st[:, 0:1], scale=1.0,
                                         accum_out=st[:, 4:5])
                else:
                    # combined bias Mn = min(m1n, m2n); w1 = exp(Mn - m1n)
                    nc.vector.tensor_tensor(out=st[:, 2:3], in0=st[:, 0:1],
                                            in1=st[:, 1:2], op=ALU.min)
                    nc.scalar.activation(st[:, 3:4], st[:, 0:1], AF.Exp,
                                         bias=st[:, 2:3], scale=-1.0)
                    nc.scalar.activation(pc[:, c0:c1], sc[:, 0:cl], AF.Exp,
                                         bias=st[:, 2:3], scale=1.0,
                                         accum_out=st[:, 5:6])
            pt = ptpool.tile([P, QT, P], f16, tag="pt", name="pt")
            ptt[(qt, h)] = pt
            nc.sync.dma_start_transpose(pt[:, 0:klen // P, :], pc[:])
            if len(chs) == 2:
                # z = z1*w1 + z2; s1 = w1*rh (w1 correction folded into the
                # per-chunk pv combine in beta — w1 is per-query-row, so it
                # can only be applied in q-partition-major layouts)
                nc.vector.tensor_scalar(out=st[:, 8:9], in0=st[:, 4:5],
                                        scalar1=st[:, 3:4], scalar2=None, op0=ALU.mult)
                nc.vector.tensor_tensor(out=st[:, 9:10], in0=st[:, 8:9],
                                        in1=st[:, 5:6], op=ALU.add)
                nc.vector.reciprocal(st[:, 6:7], st[:, 9:10])
                nc.vector.tensor_scalar(out=st[:, 7:8], in0=st[:, 6:7],
                                        scalar1=st[:, 3:4], scalar2=None, op0=ALU.mult)
            else:
                nc.vector.reciprocal(st[:, 6:7], st[:, 4:5])

        def beta(qt, h):
            if h == 0:
                ostg[qt] = ostgp.tile([P, DL], f16, tag="ostg", name=f"ostg{qt}")
            st = stats[(qt, h)]
            pt = ptt[(qt, h)]
            nkb = qt + 1
            osl = ostg[qt][:, h * DK:(h + 1) * DK]
            pv = pvp.tile([P, 2 * DK], f32, tag="pv", name="pv")
            if nkb <= CHUNK // P:
                for kb in range(nkb):
                    nc.tensor.matmul(pv[:, 0:DK], pt[:, kb, :],
                                     vsb[kb][:, h * DK:(h + 1) * DK],
                                     start=(kb == 0), stop=(kb == nkb - 1))
                nc.vector.tensor_scalar(out=osl, in0=pv[:, 0:DK],
                                        scalar1=st[:, 6:7], scalar2=None, op0=ALU.mult)
            else:
                # two accumulation groups (chunk1 / chunk2) in one psum bank;
                # combine: ostg = pv1*(w1*rh) + pv2*rh
                nb1 = CHUNK // P
                for kb in range(nb1):
                    nc.tensor.matmul(pv[:, 0:DK], pt[:, kb, :],
                                     vsb[kb][:, h * DK:(h + 1) * DK],
                                     start=(kb == 0), stop=(kb == nb1 - 1))
                for kb in range(nb1, nkb):
                    nc.tensor.matmul(pv[:, DK:2 * DK], pt[:, kb, :],
                                     vsb[kb][:, h * DK:(h + 1) * DK],
                                     start=(kb == nb1), stop=(kb == nkb - 1))
                nc.vector.tensor_scalar(out=osl, in0=pv[:, DK:2 * DK],
                                        scalar1=st[:, 6:7], scalar2=None, op0=ALU.mult)
                nc.vector.scalar_tensor_tensor(
                    out=osl, in0=pv[:, 0:DK], scalar=st[:, 7:8], in1=osl,
                    op0=ALU.mult, op1=ALU.add)
            del stats[(qt, h)], ptt[(qt, h)]

        def finish_qt(qt):
            ot = outtp.tile([P, NT, P], f16, tag="outT", name=f"outT{qt}")
            nc.sync.dma_start_transpose(ot[:], ostg[qt][:])
            for n2 in range(2):
                ps = mmp.tile([P, 512], f32, tag="mm512", name=f"psy{qt}{n2}")
                for j in range(NT):
                    nc.tensor.matmul(ps[:], ot[:, j, :],
                                     wo_sb[:, j, n2 * 512:(n2 + 1) * 512],
                                     start=(j == 0), stop=(j == NT - 1))
                ysb = ypool.tile([P, 512], f32, tag="y", name="ysb")
                nc.vector.tensor_copy(ysb[:], ps[:])
                nc.gpsimd.dma_start(
                    out=y[qt * P:(qt + 1) * P, n2 * 512:(n2 + 1) * 512], in_=ysb[:])
            del ostg[qt]

        # ---------- interleaved emission ----------
        iters = [(qt, h) for qt in range(QT) for h in range(HL)]
        proj_group(0)
        for i, (qt, h) in enumerate(iters):
            alpha(qt, h)
            # kick off next projection slab 1/4 into each attention group
            if h == HL - 1 and qt % 4 == 0 and qt // 4 + 1 < MT:
                proj_group(qt // 4 + 1)
            if i >= LAG:
                bqt, bh = iters[i - LAG]
                beta(bqt, bh)
                if bh == HL - 1:
                    finish_qt(bqt)
        for j in range(len(iters) - LAG, len(iters)):
            bqt, bh = iters[j]
            beta(bqt, bh)
            if bh == HL - 1:
                finish_qt(bqt)


def _host_prep(q, k, v, wq, wk, wv, wo):
    """Build the 8 per-core input maps."""
    def split16(x):
        hi = x.astype(np.float16)
        lo = (x - hi.astype(np.float32)).astype(np.float16)
        return hi, lo

    scale = np.float32(1.0 / np.sqrt(DK))
    maskp = np.zeros((P, CHUNK), np.float32)
    maskp[:, CHUNK - P:] = np.triu(np.full((P, P), -1e30, np.float32), k=1)
    in_maps = []
    per_b = {}
    for b in range(B):
        per_b[b] = (np.ascontiguousarray(q[b].T).astype(np.float16),
                    np.ascontiguousarray(k[b].T).astype(np.float16),
                    np.ascontiguousarray(v[b].T).astype(np.float16))
    per_g = {}
    for g in range(2):
        cs = slice(g * DL, (g + 1) * DL)
        wq_h, wq_l = split16(np.ascontiguousarray(wq[:, cs]) * scale)
        wk_h, wk_l = split16(np.ascontiguousarray(wk[:, cs]))
        per_g[g] = (wq_h, wq_l, wk_h, wk_l,
                    np.ascontiguousarray(wv[:, cs]).astype(np.float16),
                    np.ascontiguousarray(wo[cs, :]).astype(np.float16))
    for c in range(N_CORES):
        b, g = c // 2, c % 2
        xq, xk, xv = per_b[b]
        wq_h, wq_l, wk_h, wk_l, wv_c, wo_c = per_g[g]
        in_maps.append({
            "xq16": xq, "xk16": xk, "xv16": xv,
            "wq_hi": wq_h, "wq_lo": wq_l, "wk_hi": wk_h, "wk_lo": wk_l,
            "wv16": wv_c, "wo16": wo_c, "maskp": maskp,
        })
    return in_maps


def kernel(q, k, v, wq, wk, wv, wo):
    if "nc" not in _cache:
        _cache["nc"] = _build()
    nc = _cache["nc"]
    in_maps = _host_prep(np.asarray(q), np.asarray(k), np.asarray(v),
                         np.asarray(wq), np.asarray(wk), np.asarray(wv),
                         np.asarray(wo))
    res = run_bass_kernel_spmd(nc, in_maps, list(range(N_CORES)))
    out = np.empty((B, S, D), np.float32)
    for b in range(B):
        out[b] = res.results[2 * b]["y"] + res.results[2 * b + 1]["y"]
    return out


if __name__ == "__main__":
    d = np.load("/root/problem/inputs_cache.npz")
    out = kernel(d["q"], d["k"], d["v"], d["wq"], d["wk"], d["wv"], d["wo"])
    ref = d["ref"]
    rel = np.linalg.norm(out - ref) / np.linalg.norm(ref)
    print(f"Relative error: {rel:.4e}")
